# revision 5
# baseline (speedup 1.0000x reference)
"""Trainium2 Bass kernel for nn_MoEExpertPool (MoE product-of-experts).

Math (per reference):
  3 modality groups (fs, cb, sp) x 4 experts each = 12 experts.
  Per expert e: h = relu(x @ W1_e + b1_e); mu_e = h @ Wmu_e + bmu_e;
                lv_e = h @ Wlv_e + blv_e.
  Gate per group: w = softmax(x @ Wg + bg) (cb additionally scaled by
  (1 - mean(modality_mask))).  PoE fuse over the 12 experts:
    prec_e = 1 / (exp(lv_e) + eps)  ~= exp(-lv_e)   (eps negligible)
    S2 = sum_e prec_e ; S1 = sum_e w_e * mu_e * prec_e
    mu_fused = S1 / S2 ; logvar_fused = log(1/S2 + eps)

Sharding: pure batch-parallel over 8 cores (512 rows each); every core runs
all 12 experts so S1/S2 are complete per-core — no cross-core reduction.
Device works in a transposed layout (contraction dim on partitions), so
matmuls chain with no on-chip transposes and the per-column biases become
per-partition activation biases.  Matmul operands are bf16 (~3.6e-3 rel
err end-to-end, well under the 2e-2 gate; fp8 fails at 4e-2+ even on the
lv path only) which halves weight DMA vs fp32r (290 MB/core/exec, fully
hidden under PE work).  The kernel is tensor-engine bound; sustained
throughput sits at the P0 power-state rate (~2.0 GHz effective, ~265 ns
per 128x512 matmul) rather than the 2.4 GHz paper rate — measured via
SBUF-only matmul microbenchmarks, and insensitive to LDWEIGHTS
amortization.  Experts are software-pipelined (l1 of expert e+1 issues
before l2 of expert e; h pool holds 2 buffers) so the PE never waits on
the relu chain at expert boundaries.  Weights are pre-tiled on host to
[mat, mt, p, kt, m] so each strip DMA is 4 KB-contiguous per partition;
outputs stream per-mt during the last expert.  Gates + final divide/log
run on host (0.01% of FLOPs).
"""

import sys

sys.path.insert(0, "/opt/trn_rl_repo")

import numpy as np

B, D, E, NG = 4096, 2048, 4, 3
NEXP = NG * E            # 12 experts
N_CORES = 8
BC = B // N_CORES        # 512 batch rows per core
MT = D // 128            # 16 output tiles per matmul
KT = D // 128            # 16 contraction tiles
EPS = 1e-8

WSTRIP_BUFS = 10
H_BUFS = 2

_cache = {}


def _bcol(e, j, mt):
    # column in the packed bias tile for expert e, matrix j (0=b1,1=bmu,2=-blv)
    return (e * 3 + j) * MT + mt


def _build_nc(reps=1):
    import concourse.mybir as mybir
    import concourse.tile as tile
    from concourse import bacc

    f32 = mybir.dt.float32
    mmdt = mybir.dt.bfloat16
    AF = mybir.ActivationFunctionType

    nc = bacc.Bacc("TRN2", target_bir_lowering=False)
    xT = nc.dram_tensor("xT", [128, KT, BC], mmdt, kind="ExternalInput")
    W = nc.dram_tensor("W", [NEXP * 3, MT, 128, KT, 128], mmdt, kind="ExternalInput")
    WG = nc.dram_tensor("WG", [NEXP, BC], f32, kind="ExternalInput")
    BIAS = nc.dram_tensor("BIAS", [128, NEXP * 3 * MT], f32, kind="ExternalInput")
    S1 = nc.dram_tensor("S1", [D, BC], f32, kind="ExternalOutput")
    S2 = nc.dram_tensor("S2", [D, BC], f32, kind="ExternalOutput")

    with tile.TileContext(nc) as tc:
        with (
            tc.tile_pool(name="xp", bufs=1) as xp,
            tc.tile_pool(name="hp", bufs=H_BUFS) as hp,
            tc.tile_pool(name="accp", bufs=1) as accp,
            tc.tile_pool(name="wp", bufs=WSTRIP_BUFS) as wp,
            tc.tile_pool(name="gp", bufs=2) as gp,
            tc.tile_pool(name="cp", bufs=1) as cp,
            tc.tile_pool(name="ew", bufs=3) as ew,
            tc.tile_pool(name="psh", bufs=2, space="PSUM") as psh,
            tc.tile_pool(name="psmu", bufs=3, space="PSUM") as psmu,
            tc.tile_pool(name="pslv", bufs=3, space="PSUM") as pslv,
        ):
            xsb = xp.tile([128, KT, BC], mmdt)
            nc.sync.dma_start(xsb[:], xT[:, :, :])
            bias_sb = cp.tile([128, NEXP * 3 * MT], f32)
            nc.sync.dma_start(bias_sb[:], BIAS[:, :])
            S1sb = accp.tile([128, MT, BC], f32)
            S2sb = accp.tile([128, MT, BC], f32)

            def l1_block(e):
                # gate row broadcast + layer 1: hT = relu(W1.T @ xT + b1)
                wg_t = gp.tile([128, BC], f32, tag="wg")
                nc.sync.dma_start(wg_t[:], WG[e : e + 1, :].partition_broadcast(128))
                h = hp.tile([128, KT, BC], mmdt, tag="h")
                for mt in range(MT):
                    wst = wp.tile([128, KT, 128], mmdt, tag="wstrip")
                    nc.sync.dma_start(wst[:], W[3 * e, mt])
                    ps = psh.tile([128, BC], f32, tag="psh")
                    for kt in range(KT):
                        nc.tensor.matmul(
                            ps[:],
                            wst[:, kt, :],
                            xsb[:, kt, :],
                            start=(kt == 0),
                            stop=(kt == KT - 1),
                        )
                    nc.scalar.activation(
                        h[:, mt, :], ps[:], AF.Relu,
                        bias=bias_sb[:, _bcol(e, 0, mt) : _bcol(e, 0, mt) + 1],
                    )
                return wg_t, h

            def l2_block(e, wg_t, h, store):
                # layer 2: muT, lvT; fold into PoE partial sums
                for mt in range(MT):
                    wmu = wp.tile([128, KT, 128], mmdt, tag="wstrip")
                    nc.sync.dma_start(wmu[:], W[3 * e + 1, mt])
                    wlv = wp.tile([128, KT, 128], mmdt, tag="wstrip")
                    nc.sync.dma_start(wlv[:], W[3 * e + 2, mt])
                    pmu = psmu.tile([128, BC], f32, tag="pmu")
                    plv = pslv.tile([128, BC], f32, tag="plv")
                    for kt in range(KT):
                        nc.tensor.matmul(
                            pmu[:], wmu[:, kt, :], h[:, kt, :],
                            start=(kt == 0), stop=(kt == KT - 1),
                        )
                    for kt in range(KT):
                        nc.tensor.matmul(
                            plv[:], wlv[:, kt, :], h[:, kt, :],
                            start=(kt == 0), stop=(kt == KT - 1),
                        )
                    # prec = exp(-(plv + blv)); expert 0 writes, rest accumulate
                    blv_col = bias_sb[:, _bcol(e, 2, mt) : _bcol(e, 2, mt) + 1]
                    if e == 0:
                        prec_dst = S2sb[:, mt, :]
                    else:
                        prec_t = ew.tile([128, BC], f32, tag="prec")
                        prec_dst = prec_t[:]
                    nc.scalar.activation(
                        prec_dst, plv[:], AF.Exp, bias=blv_col, scale=-1.0,
                    )
                    if e > 0:
                        nc.vector.tensor_add(S2sb[:, mt, :], S2sb[:, mt, :], prec_dst)
                    precw = ew.tile([128, BC], f32, tag="precw")
                    nc.vector.tensor_mul(precw[:], prec_dst, wg_t[:])
                    if e == 0:
                        nc.vector.scalar_tensor_tensor(
                            S1sb[:, mt, :], pmu[:],
                            bias_sb[:, _bcol(e, 1, mt) : _bcol(e, 1, mt) + 1],
                            precw[:],
                            op0=mybir.AluOpType.add, op1=mybir.AluOpType.mult,
                        )
                    else:
                        mu = ew.tile([128, BC], f32, tag="mu")
                        nc.vector.scalar_tensor_tensor(
                            mu[:], pmu[:],
                            bias_sb[:, _bcol(e, 1, mt) : _bcol(e, 1, mt) + 1],
                            precw[:],
                            op0=mybir.AluOpType.add, op1=mybir.AluOpType.mult,
                        )
                        nc.vector.tensor_add(S1sb[:, mt, :], S1sb[:, mt, :], mu[:])
                    if store:
                        nc.sync.dma_start(
                            S1[mt * 128 : (mt + 1) * 128, :], S1sb[:, mt, :]
                        )
                        nc.sync.dma_start(
                            S2[mt * 128 : (mt + 1) * 128, :], S2sb[:, mt, :]
                        )

            # reps>1 builds a timing variant that repeats the whole
            # computation; only the last rep's outputs are stored.
            # Experts are software-pipelined: l1(e+1) is emitted before
            # l2(e) so the PE never waits on the h-relu chain at expert
            # boundaries (h pool holds 2 live buffers).
            for rep in range(reps):
                last = None
                for e in range(NEXP):
                    cur = (e, *l1_block(e))
                    if last is not None:
                        le, lw, lh = last
                        l2_block(le, lw, lh, store=False)
                    last = cur
                le, lw, lh = last
                l2_block(le, lw, lh, store=(rep == reps - 1))

    nc.compile()
    return nc


def _get_nc(reps=1):
    key = ("nc", reps)
    if key not in _cache:
        _cache[key] = _build_nc(reps)
    return _cache[key]


def _host_prep(inputs):
    import ml_dtypes

    mmdt_np = ml_dtypes.bfloat16

    x = np.asarray(inputs["x"], np.float32)
    mask = np.asarray(inputs["modality_mask"])
    xd = x.astype(np.float64)
    mask_mean = mask.astype(np.float64).mean(axis=1, keepdims=True)  # [B,1]

    prefs = ["fs", "cb", "sp"]
    # gate weights [NEXP, B]
    wgate = np.empty((NEXP, B), np.float32)
    for g, pref in enumerate(prefs):
        logits = xd @ np.asarray(inputs[f"{pref}_Wg"], np.float64) + np.asarray(
            inputs[f"{pref}_bg"], np.float64
        )
        logits -= logits.max(axis=1, keepdims=True)
        ex = np.exp(logits)
        w = ex / ex.sum(axis=1, keepdims=True)  # [B, E]
        if pref == "cb":
            w = w * (1.0 - mask_mean)
        wgate[g * E : (g + 1) * E, :] = w.T.astype(np.float32)

    # weights pre-tiled to [mat, mt, p, kt, m]
    Wstack = np.empty((NEXP * 3, MT, 128, KT, 128), mmdt_np)
    bias_arr = np.zeros((128, NEXP * 3 * MT), np.float32)
    for g, pref in enumerate(prefs):
        for e in range(E):
            ge = g * E + e
            for j, nm in enumerate(["W1", "Wmu", "Wlv"]):
                Wf = np.asarray(inputs[f"{pref}_{nm}"][e], np.float32)  # [D, D]
                Wstack[ge * 3 + j] = (
                    Wf.reshape(KT, 128, MT, 128).transpose(2, 1, 0, 3).astype(mmdt_np)
                )
            for j, nm in enumerate(["b1", "bmu", "blv"]):
                vec = np.asarray(inputs[f"{pref}_{nm}"][e], np.float32)  # [D]
                if nm == "blv":
                    vec = -vec  # negated: prec = exp(-(plv + blv))
                bias_arr[:, (ge * 3 + j) * MT : (ge * 3 + j + 1) * MT] = vec.reshape(
                    MT, 128
                ).T

    in_maps = []
    for c in range(N_CORES):
        xc = x.T[:, c * BC : (c + 1) * BC]  # [D, BC]
        xt2 = np.ascontiguousarray(
            xc.reshape(KT, 128, BC).transpose(1, 0, 2).astype(mmdt_np)
        )
        in_maps.append(
            {
                "xT": xt2,
                "W": Wstack,
                "WG": np.ascontiguousarray(wgate[:, c * BC : (c + 1) * BC]),
                "BIAS": bias_arr,
            }
        )
    return in_maps


def _finalize(results):
    S1 = np.concatenate([r["S1"] for r in results], axis=1)  # [D, B]
    S2 = np.concatenate([r["S2"] for r in results], axis=1)  # [D, B]
    S2d = S2.astype(np.float64)
    mu_fused = (S1.astype(np.float64) / S2d).T.astype(np.float32)
    logvar_fused = np.log(1.0 / S2d + EPS).T.astype(np.float32)
    return mu_fused, logvar_fused


def kernel(run_kwargs=None, **inputs):
    from concourse.bass_utils import run_bass_kernel_spmd

    nc = _get_nc()
    in_maps = _host_prep(inputs)
    res = run_bass_kernel_spmd(
        nc, in_maps, core_ids=list(range(N_CORES)), **(run_kwargs or {})
    )
    _cache["last_result"] = res
    return _finalize(res.results)
